# revision 1
# baseline (speedup 1.0000x reference)
"""Trainium2 Bass kernel for nn_NodeEmbDecoder (LSTM decoder + masked NN assignment).

Sharding: data-parallel over batch B=256 across 8 cores (32 rows each),
weights replicated and SBUF-resident. All activations kept transposed
([feature -> partitions, batch -> free]) so gate matmuls use weight tiles as
the stationary operand and the 32-row batch slice as the moving operand.

Everything is fp32 (true 4-pass PE matmuls) because the masked argmin over
node distances is decision-sensitive: lower precision flips assignments and
permutes whole output rows.
"""

import numpy as np

B, E, D, H, L, N = 256, 512, 128, 512, 2, 128
NCORES = 8
BL = B // NCORES  # 32 batch rows per core

_CACHE = {}


def _build(reps=1, debug=False):
    import concourse.bass as bass
    import concourse.bacc as bacc
    import concourse.tile as tile
    from concourse import mybir
    from concourse.bass import ds

    fp32 = mybir.dt.float32
    AF = mybir.ActivationFunctionType
    ALU = mybir.AluOpType

    nc = bacc.Bacc(None, target_bir_lowering=False, debug=True)

    # ---- DRAM I/O (per-core slices, host pre-laid-out) ----
    d_embT = nc.dram_tensor("embT", [128, 4, BL], fp32, kind="ExternalInput")
    d_nodeT = nc.dram_tensor("nodeT", [128, BL, N], fp32, kind="ExternalInput")
    d_W1 = nc.dram_tensor("W1T", [128, 4 * H], fp32, kind="ExternalInput")
    d_W2 = nc.dram_tensor("W2T", [128, 4 * 2 * H], fp32, kind="ExternalInput")
    d_Wih0 = nc.dram_tensor("Wih0T", [128, 1 * 4 * H], fp32, kind="ExternalInput")
    d_Whh0 = nc.dram_tensor("Whh0T", [128, 4 * 4 * H], fp32, kind="ExternalInput")
    d_Wih1 = nc.dram_tensor("Wih1T", [128, 4 * 4 * H], fp32, kind="ExternalInput")
    d_Whh1 = nc.dram_tensor("Whh1T", [128, 4 * 4 * H], fp32, kind="ExternalInput")
    d_Wo = nc.dram_tensor("WoT", [128, 4 * D], fp32, kind="ExternalInput")
    d_b1 = nc.dram_tensor("b1c", [128, 4], fp32, kind="ExternalInput")
    d_b2 = nc.dram_tensor("b2c", [128, 8], fp32, kind="ExternalInput")
    d_b0g = nc.dram_tensor("b0g", [128, 16], fp32, kind="ExternalInput")
    d_b1g = nc.dram_tensor("b1g", [128, 16], fp32, kind="ExternalInput")
    d_bo = nc.dram_tensor("boc", [128, 1], fp32, kind="ExternalInput")
    d_iota = nc.dram_tensor("iota128", [128, N], fp32, kind="ExternalInput")

    d_out = nc.dram_tensor("outT", [BL, N, D], fp32, kind="ExternalOutput")
    d_idx = nc.dram_tensor("idxs", [BL, N], fp32, kind="ExternalOutput")
    if debug:
        d_preds = nc.dram_tensor("predsD", [128, N * BL], fp32, kind="ExternalOutput")
        d_scores = nc.dram_tensor("scoresD", [BL, N * N], fp32, kind="ExternalOutput")

    with tile.TileContext(nc) as tc:
        import contextlib

        with contextlib.ExitStack() as ctx:
            wp = ctx.enter_context(tc.tile_pool(name="wp", bufs=1))
            st = ctx.enter_context(tc.tile_pool(name="st", bufs=1))
            ga = ctx.enter_context(tc.tile_pool(name="ga", bufs=2))
            ps = ctx.enter_context(tc.tile_pool(name="ps", bufs=6, space="PSUM"))
            ps2 = ctx.enter_context(tc.tile_pool(name="ps2", bufs=2, space="PSUM"))
            sc = ctx.enter_context(tc.tile_pool(name="sc", bufs=4))
            dr = ctx.enter_context(tc.tile_pool(name="dr", bufs=1, space="DRAM"))

            # ---- weights / constants into SBUF (once) ----
            W1 = wp.tile([128, 4, H], fp32)
            W2 = wp.tile([128, 4, 2 * H], fp32)
            Wih0 = wp.tile([128, 1, 4 * H], fp32)
            Whh0 = wp.tile([128, 4, 4 * H], fp32)
            Wih1 = wp.tile([128, 4, 4 * H], fp32)
            Whh1 = wp.tile([128, 4, 4 * H], fp32)
            Wo = wp.tile([128, 4, D], fp32)
            embT = wp.tile([128, 4, BL], fp32)
            nodeT = wp.tile([128, BL, N], fp32)
            b1c = wp.tile([128, 4], fp32)
            b2c = wp.tile([128, 8], fp32)
            b0g = wp.tile([128, 16], fp32)
            b1g = wp.tile([128, 16], fp32)
            boc = wp.tile([128, 1], fp32)
            iota = wp.tile([128, N], fp32)
            for dst, src in [
                (W1, d_W1), (W2, d_W2), (Wih0, d_Wih0), (Whh0, d_Whh0),
                (Wih1, d_Wih1), (Whh1, d_Whh1), (Wo, d_Wo), (embT, d_embT),
                (nodeT, d_nodeT), (b1c, d_b1), (b2c, d_b2), (b0g, d_b0g),
                (b1g, d_b1g), (boc, d_bo), (iota, d_iota),
            ]:
                nc.sync.dma_start(out=dst[:], in_=src[:])

            # persistent state
            h0S = st.tile([128, 4, BL], fp32)
            h1S = st.tile([128, 4, BL], fp32)
            c0S = st.tile([128, 4, BL], fp32)
            c1S = st.tile([128, 4, BL], fp32)
            xS = st.tile([128, BL], fp32)
            rT = st.tile([128, 4, BL], fp32)
            predsT = st.tile([128, N * BL], fp32)  # [d, t*BL + b]
            scoresQ = dr.tile([BL, N * N], fp32)  # DRAM scratch [b, t*N + n]
            msk = st.tile([BL, N], fp32)
            idxs = st.tile([BL, N], fp32)
            onesK = st.tile([128, 1], fp32)
            onesR = st.tile([1, N], fp32)
            ident = st.tile([128, 128], fp32)
            idxsT = st.tile([128, BL], fp32)

            nc.vector.memset(onesK[:], 1.0)
            nc.vector.memset(onesR[:], 1.0)
            from concourse.masks import make_identity
            make_identity(nc, ident[:])

            predsR = predsT[:].rearrange("p (t b) -> p t b", b=BL)

            import contextlib as _cl
            _loop = tc.For_i(0, reps, 1) if reps > 1 else _cl.nullcontext()
            with _loop:
                nc.vector.memset(c0S[:], 0.0)
                nc.vector.memset(c1S[:], 0.0)
                nc.vector.memset(xS[:], 0.0)
                nc.vector.memset(msk[:], 0.0)

                # ---- stage 1: FNN_in -> h0, h1 ----
                for m in range(4):
                    pt = ps.tile([128, BL], fp32, tag="ps")
                    for k in range(4):
                        nc.tensor.matmul(pt[:], W1[:, k, m * 128:(m + 1) * 128],
                                         embT[:, k, :], start=(k == 0), stop=(k == 3))
                    nc.scalar.activation(rT[:, m, :], pt[:], AF.Relu, bias=b1c[:, m:m + 1])
                for m in range(8):
                    pt = ps.tile([128, BL], fp32, tag="ps")
                    for k in range(4):
                        nc.tensor.matmul(pt[:], W2[:, k, m * 128:(m + 1) * 128],
                                         rT[:, k, :], start=(k == 0), stop=(k == 3))
                    dst = h0S[:, m, :] if m < 4 else h1S[:, m - 4, :]
                    nc.scalar.activation(dst, pt[:], AF.Identity, bias=b2c[:, m:m + 1])

                # ---- stage 2: LSTM decode, 128 steps ----
                def lstm_layer(WihS, WhhS, nih, x_aps, biasS, hS, cS):
                    # new h goes to temporaries first: hS must stay readable
                    # (old values) for all 16 m-tiles' Whh matmuls.
                    acts = {}
                    hnew = []
                    for m in range(16):
                        g, j = m // 4, m % 4
                        pt = ps.tile([128, BL], fp32, tag="ps")
                        for k in range(nih):
                            nc.tensor.matmul(pt[:], WihS[:, k, m * 128:(m + 1) * 128],
                                             x_aps[k], start=(k == 0), stop=False)
                        for k in range(4):
                            nc.tensor.matmul(pt[:], WhhS[:, k, m * 128:(m + 1) * 128],
                                             hS[:, k, :], start=False, stop=(k == 3))
                        a = ga.tile([128, BL], fp32, tag=f"a{g}{j}")
                        fn = AF.Tanh if g == 2 else AF.Sigmoid
                        nc.scalar.activation(a[:], pt[:], fn, bias=biasS[:, m:m + 1])
                        acts[(g, j)] = a
                        if g == 3:
                            i_, f_, g_, o_ = (acts[(0, j)], acts[(1, j)],
                                              acts[(2, j)], acts[(3, j)])
                            t1 = ga.tile([128, BL], fp32, tag=f"t1{j}")
                            nc.vector.tensor_tensor(t1[:], i_[:], g_[:], op=ALU.mult)
                            t2 = ga.tile([128, BL], fp32, tag=f"t2{j}")
                            nc.vector.tensor_tensor(t2[:], f_[:], cS[:, j, :], op=ALU.mult)
                            nc.vector.tensor_tensor(cS[:, j, :], t1[:], t2[:], op=ALU.add)
                            tct = ga.tile([128, BL], fp32, tag=f"tc{j}")
                            nc.scalar.activation(tct[:], cS[:, j, :], AF.Tanh)
                            hn = ga.tile([128, BL], fp32, tag=f"hn{j}")
                            nc.vector.tensor_tensor(hn[:], o_[:], tct[:], op=ALU.mult)
                            hnew.append(hn)
                    for j in range(4):
                        nc.vector.tensor_copy(hS[:, j, :], hnew[j][:])

                with tc.For_i(0, N, 1) as t:
                    lstm_layer(Wih0, Whh0, 1, [xS[:]], b0g, h0S, c0S)
                    lstm_layer(Wih1, Whh1, 4, [h0S[:, k, :] for k in range(4)],
                               b1g, h1S, c1S)
                    pt = ps.tile([128, BL], fp32, tag="ps")
                    for k in range(4):
                        nc.tensor.matmul(pt[:], Wo[:, k, :], h1S[:, k, :],
                                         start=(k == 0), stop=(k == 3))
                    nc.scalar.activation(xS[:], pt[:], AF.Identity, bias=boc[:])
                    nc.vector.tensor_copy(predsT[:, ds(t * BL, BL)], xS[:])

                # ---- stage 3: scores[b][t,n] = <pred_t, e_n> - 0.5||e_n||^2 ----
                for b in range(BL):
                    sqb = sc.tile([128, N], fp32, tag="sqb")
                    nc.scalar.activation(sqb[:], nodeT[:, b, :], AF.Square)
                    npt = ps2.tile([1, N], fp32, tag="x2")
                    nc.tensor.matmul(npt[:], onesK[:], sqb[:], start=True, stop=True)
                    neghb = sc.tile([1, N], fp32, tag="neghb")
                    nc.scalar.activation(neghb[:], npt[:], AF.Copy, scale=-0.5)
                    dpt = ps2.tile([128, N], fp32, tag="x2")
                    nc.tensor.matmul(dpt[:], onesR[:], neghb[:],
                                     start=True, stop=False)
                    nc.tensor.matmul(dpt[:], predsR[:, :, b], nodeT[:, b, :],
                                     start=False, stop=True)
                    sb = sc.tile([128, N], fp32, tag="scb")
                    nc.vector.tensor_copy(sb[:], dpt[:])
                    nc.sync.dma_start(out=scoresQ[b:b + 1, :], in_=sb[:])

                # ---- stage 4: sequential masked argmax over n ----
                mx8 = st.tile([BL, 8], fp32)
                ix8 = st.tile([BL, 8], mybir.dt.uint32)
                ixf = st.tile([BL, 1], fp32)
                mtile = st.tile([BL, N], fp32)
                eqm = st.tile([BL, N], fp32)
                for t in range(N):
                    cur = sc.tile([BL, N], fp32, tag="cur")
                    nc.sync.dma_start(out=cur[:], in_=scoresQ[0:BL, t * N:(t + 1) * N])
                    nc.vector.tensor_tensor(mtile[:], cur[:], msk[:], op=ALU.add)
                    nc.vector.max(mx8[:], mtile[:])
                    nc.vector.max_index(ix8[:], mx8[:], mtile[:])
                    nc.vector.tensor_copy(ixf[:], ix8[:, 0:1])
                    nc.vector.tensor_copy(idxs[:, t:t + 1], ixf[:])
                    nc.vector.tensor_scalar(eqm[:], iota[0:BL, :], ixf[:], -1e30,
                                            op0=ALU.is_equal, op1=ALU.mult)
                    nc.vector.tensor_tensor(msk[:], msk[:], eqm[:], op=ALU.add)

                # ---- stage 5: permute preds into output slots ----
                ipt = ps2.tile([128, BL], fp32, tag="x2")
                nc.tensor.transpose(ipt[:], idxs[:], ident[0:BL, 0:BL])
                nc.vector.tensor_copy(idxsT[:], ipt[:])
                for b in range(BL):
                    tpt = ps2.tile([128, 128], fp32, tag="x2")
                    nc.tensor.transpose(tpt[:], predsR[:, :, b], ident[:])
                    pb = sc.tile([128, 128], fp32, tag="pb")
                    nc.vector.tensor_copy(pb[:], tpt[:])
                    oh = sc.tile([128, N], fp32, tag="oh")
                    nc.vector.tensor_scalar(oh[:], iota[:], idxsT[:, b:b + 1], None,
                                            op0=ALU.is_equal)
                    opt = ps2.tile([N, D], fp32, tag="x2")
                    nc.tensor.matmul(opt[:], oh[:], pb[:], start=True, stop=True)
                    ob = sc.tile([N, D], fp32, tag="ob")
                    nc.vector.tensor_copy(ob[:], opt[:])
                    nc.sync.dma_start(out=d_out[b], in_=ob[:])

            nc.sync.dma_start(out=d_idx[:], in_=idxs[:])
            if debug:
                nc.sync.dma_start(out=d_preds[:], in_=predsT[:])
                nc.sync.dma_start(out=d_scores[:], in_=scoresQ[:])

    nc.finalize()
    return nc


def _prep_w(W):
    # torch-Linear weight [M_out, K_in] -> stationary lhsT sbuf layout
    # [128, K/128, M]:  sb[p, k, m] = W[m, k*128+p]
    M, K = W.shape
    kk = K // 128
    return np.ascontiguousarray(
        W.T.reshape(kk, 128, M).transpose(1, 0, 2).reshape(128, kk * M)
    ).astype(np.float32)


def _prep_bias_cols(b):
    # [M] -> [128, M/128] with col j = b[j*128:(j+1)*128]
    return np.ascontiguousarray(b.reshape(-1, 128).T).astype(np.float32)


def prepare_in_maps(emb, node_emb_encoded, W1, b1, W2, b2,
                    Wih0, Whh0, bih0, bhh0, Wih1, Whh1, bih1, bhh1,
                    Wo, bo):
    shared = {
        "W1T": _prep_w(np.asarray(W1)).reshape(128, 4, H),
        "W2T": _prep_w(np.asarray(W2)).reshape(128, 4, 2 * H).reshape(128, 8 * H),
        "Wih0T": _prep_w(np.asarray(Wih0)),
        "Whh0T": _prep_w(np.asarray(Whh0)),
        "Wih1T": _prep_w(np.asarray(Wih1)),
        "Whh1T": _prep_w(np.asarray(Whh1)),
        "WoT": _prep_w(np.asarray(Wo)),
        "b1c": _prep_bias_cols(np.asarray(b1)),
        "b2c": _prep_bias_cols(np.asarray(b2)),
        "b0g": _prep_bias_cols(np.asarray(bih0) + np.asarray(bhh0)),
        "b1g": _prep_bias_cols(np.asarray(bih1) + np.asarray(bhh1)),
        "boc": np.asarray(bo).reshape(128, 1).astype(np.float32),
        "iota128": np.tile(np.arange(N, dtype=np.float32), (128, 1)),
    }
    in_maps = []
    for c in range(NCORES):
        sl = slice(c * BL, (c + 1) * BL)
        emb_sl = np.asarray(emb[sl], dtype=np.float32)
        node_sl = np.asarray(node_emb_encoded[sl], dtype=np.float32)
        m = dict(shared)
        m["embT"] = np.ascontiguousarray(
            emb_sl.T.reshape(4, 128, BL).transpose(1, 0, 2))
        m["nodeT"] = np.ascontiguousarray(node_sl.transpose(2, 0, 1))
        in_maps.append(m)
    return in_maps


def run(inputs, reps=1, debug=False):
    from concourse.bass_utils import run_bass_kernel_spmd
    key = (reps, debug)
    if key not in _CACHE:
        _CACHE[key] = _build(reps=reps, debug=debug)
    nc = _CACHE[key]
    in_maps = prepare_in_maps(
        inputs["emb"], inputs["node_emb_encoded"], inputs["W1"], inputs["b1"],
        inputs["W2"], inputs["b2"], inputs["Wih0"], inputs["Whh0"],
        inputs["bih0"], inputs["bhh0"], inputs["Wih1"], inputs["Whh1"],
        inputs["bih1"], inputs["bhh1"], inputs["Wo"], inputs["bo"])
    res = run_bass_kernel_spmd(nc, in_maps, list(range(NCORES)))
    return res.results


def kernel(**inputs) -> np.ndarray:
    results = run(inputs, reps=1, debug=False)
    out = np.concatenate([r["outT"] for r in results], axis=0)
    return out.astype(np.float32)



# revision 2
# speedup vs baseline: 3.8594x; 3.8594x over previous
"""Trainium2 Bass kernel v3 for nn_NodeEmbDecoder (LSTM decoder + masked NN).

Data-parallel over batch B=256 across 8 cores (BL=32 rows each); weights
replicated in SBUF. Activations transposed: [feature->partitions,
batch->free]; 128x128 weight tiles stationary.

Core trick vs v2: fp32 matmuls at moving-width 32 cost ~437ns on TRN2
(stationary reload dominates) while bf16/fp16 cost ~37ns, and a 64-wide
moving operand costs the same as 32. So every fp32 gate matmul becomes
two fp16 matmuls via an error-compensated hi/lo split:

    x @ W ~= [x_hi | x_lo] @ W_hi   (one 64-col matmul, two products)
           +  x_hi @ W_lo           (one 32-col matmul, same PSUM cols)

The dropped x_lo@W_lo term is ~2^-22 relative; measured end-to-end pred
error 6.3e-7 == the fp32 baseline's, with 0/32768 masked-argmin flips.
The elementwise/activation tail, FNN init, and the output permutation
stay fp32.

Structure per step (PE program order, software-pipelined so PE never
waits on the act/elementwise tail):
    [L0ih_t][L1hh_t][L1ih_t][L0hh_{t+1}][Wo_t]
Gate activations batched [128,4*BL] per gate function; biases pre-added
on DVE from host-expanded tiles. Scores/argmax/permute as in v2 (PE
transpose + [t][b][n] DRAM staging, single-descriptor DMAs).
"""

import numpy as np

B, E, D, H, L, N = 256, 512, 128, 512, 2, 128
NCORES = 8
BL = B // NCORES  # 32 batch rows per core
UNROLL = 8

_CACHE = {}


def _build(reps=1, lsteps=N, debug=False):
    import concourse.bass as bass
    import concourse.bacc as bacc
    import concourse.tile as tile
    from concourse import mybir
    from concourse.bass import ds

    fp32 = mybir.dt.float32
    fp16 = mybir.dt.float16
    AF = mybir.ActivationFunctionType
    ALU = mybir.AluOpType

    nc = bacc.Bacc(None, target_bir_lowering=False, debug=True)

    # ---- DRAM I/O (per-core slices, host pre-laid-out) ----
    d_embT = nc.dram_tensor("embT", [128, 4, BL], fp32, kind="ExternalInput")
    d_nodeH = nc.dram_tensor("nodeH", [128, BL, N], fp16, kind="ExternalInput")
    d_nodeL = nc.dram_tensor("nodeL", [128, BL, N], fp16, kind="ExternalInput")
    d_W1 = nc.dram_tensor("W1T", [128, 4 * H], fp32, kind="ExternalInput")
    d_W2 = nc.dram_tensor("W2T", [128, 4 * 2 * H], fp32, kind="ExternalInput")
    dw16 = {}
    for nm, cols in [("Wih0", 1 * 4 * H), ("Whh0", 4 * 4 * H),
                     ("Wih1", 4 * 4 * H), ("Whh1", 4 * 4 * H), ("Wo", 4 * D)]:
        for hl in "HL":
            dw16[nm + hl] = nc.dram_tensor(nm + hl, [128, cols], fp16,
                                           kind="ExternalInput")
    d_b1 = nc.dram_tensor("b1c", [128, 4], fp32, kind="ExternalInput")
    d_b2 = nc.dram_tensor("b2c", [128, 8], fp32, kind="ExternalInput")
    d_bgx0 = nc.dram_tensor("bgx0", [128, 16 * BL], fp32, kind="ExternalInput")
    d_bgx1 = nc.dram_tensor("bgx1", [128, 16 * BL], fp32, kind="ExternalInput")
    d_bo = nc.dram_tensor("boc", [128, 1], fp32, kind="ExternalInput")
    d_biasBN = nc.dram_tensor("biasBN", [BL, N], fp32, kind="ExternalInput")
    d_iota = nc.dram_tensor("iota128", [128, N], fp32, kind="ExternalInput")

    d_out = nc.dram_tensor("outT", [BL, N, D], fp32, kind="ExternalOutput")
    d_idx = nc.dram_tensor("idxs", [BL, N], fp32, kind="ExternalOutput")

    NBLK = 32  # score-block width in steps
    n_blocks = lsteps // NBLK if lsteps >= NBLK else 0

    with tile.TileContext(nc) as tc:
        import contextlib

        with contextlib.ExitStack() as ctx:
            wp = ctx.enter_context(tc.tile_pool(name="wp", bufs=1))
            st = ctx.enter_context(tc.tile_pool(name="st", bufs=1))
            pg = ctx.enter_context(tc.tile_pool(name="pg", bufs=1, space="PSUM"))
            ps2 = ctx.enter_context(tc.tile_pool(name="ps2", bufs=2, space="PSUM"))
            gs = ctx.enter_context(tc.tile_pool(name="gs", bufs=2))
            sc = ctx.enter_context(tc.tile_pool(name="sc", bufs=4))
            dr = ctx.enter_context(tc.tile_pool(name="dr", bufs=1, space="DRAM"))

            # ---- weights / constants into SBUF (once) ----
            W1 = wp.tile([128, 4, H], fp32)
            W2 = wp.tile([128, 4, 2 * H], fp32)
            w16 = {}
            for nm, kk in [("Wih0", 1), ("Whh0", 4), ("Wih1", 4), ("Whh1", 4)]:
                for hl in "HL":
                    w16[nm + hl] = wp.tile([128, kk, 4 * H], fp16,
                                           name="sb" + nm + hl)
            for hl in "HL":
                w16["Wo" + hl] = wp.tile([128, 4, D], fp16, name="sbWo" + hl)
            embT = wp.tile([128, 4, BL], fp32)
            nodeH = wp.tile([128, BL, N], fp16)
            nodeL = wp.tile([128, BL, N], fp16)
            b1c = wp.tile([128, 4], fp32)
            b2c = wp.tile([128, 8], fp32)
            bgx0 = wp.tile([128, 4, 4, BL], fp32)
            bgx1 = wp.tile([128, 4, 4, BL], fp32)
            boc = wp.tile([128, 1], fp32)
            biasBN = wp.tile([BL, N], fp32)
            iota = wp.tile([128, N], fp32)
            loads = [
                (W1, d_W1), (W2, d_W2), (embT, d_embT), (nodeH, d_nodeH),
                (nodeL, d_nodeL), (b1c, d_b1), (b2c, d_b2), (bgx0, d_bgx0),
                (bgx1, d_bgx1), (boc, d_bo), (biasBN, d_biasBN), (iota, d_iota),
            ] + [(w16[k], dw16[k]) for k in w16]
            for dst, src in loads:
                nc.sync.dma_start(out=dst[:], in_=src[:])

            # persistent state
            h0S = st.tile([128, 4, BL], fp32)
            h1S = st.tile([128, 4, BL], fp32)
            c0S = st.tile([128, 4, BL], fp32)
            c1S = st.tile([128, 4, BL], fp32)
            xS = st.tile([128, BL], fp32)
            rT = st.tile([128, 4, BL], fp32)
            # hi/lo fp16 splits of matmul operands (hl-major free layout)
            h0HL = st.tile([128, 4, 2, BL], fp16)
            h1HL = st.tile([128, 4, 2, BL], fp16)
            xHL = st.tile([128, 2, BL], fp16)
            predsHL = st.tile([128, 2, N * BL], fp16)  # [d, hl, t*BL + b]
            scoresQ = dr.tile([N, BL * N], fp32)  # DRAM scratch [t][b*N + n]
            msk = st.tile([BL, N], fp32)
            idxs = st.tile([BL, N], fp32)
            ident = st.tile([128, 128], fp32)
            ident16 = st.tile([128, 128], fp16)
            idxsT = st.tile([128, BL], fp32)
            mx8 = st.tile([BL, 8], fp32)
            ix8 = st.tile([BL, 8], mybir.dt.uint32)
            mtile = st.tile([BL, N], fp32)
            eqm = st.tile([BL, N], fp32)

            from concourse.masks import make_identity
            make_identity(nc, ident[:])
            nc.scalar.activation(ident16[:], ident[:], AF.Copy)

            predsHLR = predsHL[:].rearrange("p h (t b) -> p h t b", b=BL)
            hS = [h0S, h1S]
            cS = [c0S, c1S]
            hHL = [h0HL, h1HL]
            WihH = [w16["Wih0H"], w16["Wih1H"]]
            WihL = [w16["Wih0L"], w16["Wih1L"]]
            WhhH = [w16["Whh0H"], w16["Whh1H"]]
            WhhL = [w16["Whh0L"], w16["Whh1L"]]
            bgxS = [bgx0, bgx1]
            nih = [1, 4]

            def split16(dst_hl, src, tmp_tag, shape_cols):
                # dst_hl[...,0,:]=fp16(src); dst_hl[...,1,:]=fp16(src-hi)
                hi = dst_hl[:, 0, :] if len(dst_hl.shape) == 3 else dst_hl[:, :, 0, :]
                lo = dst_hl[:, 1, :] if len(dst_hl.shape) == 3 else dst_hl[:, :, 1, :]
                nc.scalar.activation(hi, src, AF.Copy)
                t32 = gs.tile(shape_cols, fp32, tag=tmp_tag)
                nc.scalar.activation(t32[:], hi, AF.Copy)
                nc.vector.tensor_tensor(lo, src, t32[:], op=ALU.subtract)

            pend = {0: None, 1: None}

            def gates_hh(l):
                # One merged accumulation group per 2KB PSUM bank (start=True
                # zeroes the whole bank): bank A holds gates 0-1, bank B 2-3.
                pt = pg.tile([128, 4, 4, 2 * BL], fp32, tag=f"P{l}")
                for g in range(4):
                    for j in range(4):
                        m = 4 * g + j
                        sl = slice(m * 128, (m + 1) * 128)
                        for kk in range(4):
                            mv = hHL[l][:, kk, :, :]
                            st_ = (g in (0, 2) and j == 0 and kk == 0)
                            nc.tensor.matmul(pt[:, g, j, :], WhhH[l][:, kk, sl],
                                             mv, start=st_, stop=False)
                            nc.tensor.matmul(pt[:, g, j, :], WhhL[l][:, kk, sl],
                                             mv, start=False, stop=False)
                pend[l] = pt

            def gates_ih_act(l):
                # finish bank groups with ih chunks, fold lo columns, bias, act
                pt = pend[l]
                for g in range(4):
                    for j in range(4):
                        m = 4 * g + j
                        sl = slice(m * 128, (m + 1) * 128)
                        for kk in range(nih[l]):
                            mv = xHL[:, :, :] if l == 0 else hHL[0][:, kk, :, :]
                            last = (g in (1, 3) and j == 3 and kk == nih[l] - 1)
                            nc.tensor.matmul(pt[:, g, j, :], WihH[l][:, kk, sl],
                                             mv, start=False, stop=False)
                            nc.tensor.matmul(pt[:, g, j, :], WihL[l][:, kk, sl],
                                             mv, start=False, stop=last)
                # TensorTensor may read at most one PSUM input: route the
                # hi+lo fold through an SBUF tile, folding the bias in too.
                gt = gs.tile([128, 4, 4, BL], fp32, tag=f"gt{l}")
                nc.vector.tensor_tensor(gt[:], pt[:, :, :, 0:BL], bgxS[l][:],
                                        op=ALU.add)
                nc.vector.tensor_tensor(gt[:], gt[:], pt[:, :, :, BL:2 * BL],
                                        op=ALU.add)
                acts = []
                for g in range(4):
                    a = gs.tile([128, 4, BL], fp32, tag=f"G{l}{g}")
                    fn = AF.Tanh if g == 2 else AF.Sigmoid
                    nc.scalar.activation(a[:], gt[:, g, :, :], fn)
                    acts.append(a)
                return acts

            def lstm_tail(l):
                acts = gates_ih_act(l)
                gi, gf, gg, go = acts
                t1 = gs.tile([128, 4, BL], fp32, tag=f"t1{l}")
                nc.vector.tensor_tensor(t1[:], gi[:], gg[:], op=ALU.mult)
                t2 = gs.tile([128, 4, BL], fp32, tag=f"t2{l}")
                nc.vector.tensor_tensor(t2[:], gf[:], cS[l][:], op=ALU.mult)
                nc.vector.tensor_tensor(cS[l][:], t1[:], t2[:], op=ALU.add)
                tct = gs.tile([128, 4, BL], fp32, tag=f"tc{l}")
                nc.scalar.activation(tct[:], cS[l][:], AF.Tanh)
                nc.vector.tensor_tensor(hS[l][:], go[:], tct[:], op=ALU.mult)
                split16(hHL[l][:], hS[l][:], f"s{l}", [128, 4, BL])

            def wo_step(col_ap):
                # hi and lo products accumulate into the same PSUM columns
                # (4 narrow matmuls per chunk) so no fold is needed.
                pt = ps2.tile([128, BL], fp32, tag="wo", bufs=1)
                for kk in range(4):
                    for wk, hl in [("WoH", 0), ("WoH", 1), ("WoL", 0), ("WoL", 1)]:
                        nc.tensor.matmul(pt[:], w16[wk][:, kk, :],
                                         h1HL[:, kk, hl, :],
                                         start=(kk == 0 and wk == "WoH" and hl == 0),
                                         stop=(kk == 3 and wk == "WoL" and hl == 1))
                nc.scalar.activation(xS[:], pt[:], AF.Identity, bias=boc[:])
                split16(xHL[:], xS[:], "sx", [128, BL])
                nc.vector.tensor_copy(predsHL[:, :, ds(col_ap, BL)], xHL[:])

            import contextlib as _cl
            _loop = tc.For_i(0, reps, 1) if reps > 1 else _cl.nullcontext()
            with _loop:
                nc.vector.memset(c0S[:], 0.0)
                nc.vector.memset(c1S[:], 0.0)
                nc.vector.memset(xS[:], 0.0)
                nc.vector.memset(xHL[:], 0.0)
                nc.vector.tensor_copy(msk[:], biasBN[:])

                # ---- stage 1: FNN_in -> h0, h1 (fp32) ----
                def s3buf(i):
                    return ps2.tile([128, 512], fp32, tag=f"s3{i % 2}", bufs=1,
                                    name=f"s3buf{i % 2}")

                for m in range(4):
                    pt = s3buf(m)[:, 0:BL]
                    for k in range(4):
                        nc.tensor.matmul(pt, W1[:, k, m * 128:(m + 1) * 128],
                                         embT[:, k, :], start=(k == 0), stop=(k == 3))
                    nc.scalar.activation(rT[:, m, :], pt, AF.Relu, bias=b1c[:, m:m + 1])
                for m in range(8):
                    pt = s3buf(m)[:, 0:BL]
                    for k in range(4):
                        nc.tensor.matmul(pt, W2[:, k, m * 128:(m + 1) * 128],
                                         rT[:, k, :], start=(k == 0), stop=(k == 3))
                    dst = h0S[:, m, :] if m < 4 else h1S[:, m - 4, :]
                    nc.scalar.activation(dst, pt, AF.Identity, bias=b2c[:, m:m + 1])
                split16(h0HL[:], h0S[:], "s0", [128, 4, BL])
                split16(h1HL[:], h1S[:], "s1", [128, 4, BL])

                # ---- stage 2: LSTM decode, software-pipelined ----
                with tc.For_i(0, lsteps, UNROLL) as t0:
                    gates_hh(0)
                    for k in range(UNROLL):
                        lstm_tail(0)
                        gates_hh(1)
                        lstm_tail(1)
                        if k < UNROLL - 1:
                            gates_hh(0)
                        wo_step(t0 * BL + k * BL)

                # ---- stage 3: score blocks -> DRAM in [t][b][n] layout ----
                for j in range(n_blocks):
                    tsl = slice(NBLK * j, NBLK * (j + 1))
                    for b in range(BL):
                        s3 = s3buf(b)
                        psc = s3[:, 0:NBLK]
                        for nk, hl in [("H", 0), ("H", 1), ("L", 0), ("L", 1)]:
                            nd = nodeH if nk == "H" else nodeL
                            nc.tensor.matmul(psc, nd[:, b, :],
                                             predsHLR[:, hl, tsl, b],
                                             start=(nk == "H" and hl == 0),
                                             stop=(nk == "L" and hl == 1))
                        ssb = sc.tile([128, NBLK], fp32, tag="ssb")
                        nc.scalar.activation(ssb[:], psc, AF.Copy)
                        pst = s3[0:NBLK, 2 * NBLK:2 * NBLK + N]
                        nc.tensor.transpose(pst, ssb[:], ident[:])
                        stb = sc.tile([NBLK, N], fp32, tag="stb")
                        nc.scalar.activation(stb[:], pst, AF.Copy)
                        nc.sync.dma_start(
                            out=scoresQ[tsl, N * b:N * (b + 1)],
                            in_=stb[:])

                # ---- stage 4: sequential masked argmax over n ----
                for t in range(lsteps if n_blocks else 0):
                    rd = sc.tile([BL, N], fp32, tag="rd")
                    nc.sync.dma_start(out=rd[:], in_=scoresQ[t:t + 1, :])
                    nc.vector.tensor_tensor(mtile[:], rd[:], msk[:], op=ALU.add)
                    nc.vector.max(mx8[:], mtile[:])
                    nc.vector.max_index(ix8[:], mx8[:], mtile[:])
                    nc.vector.tensor_copy(idxs[:, t:t + 1], ix8[:, 0:1])
                    nc.vector.tensor_scalar(eqm[:], iota[0:BL, :], idxs[:, t:t + 1],
                                            -1e30, op0=ALU.is_equal, op1=ALU.mult)
                    nc.vector.tensor_tensor(msk[:], msk[:], eqm[:], op=ALU.add)

                # ---- stage 5: permute preds into output slots (fp32) ----
                s5i = ps2.tile([128, 512], fp32, tag="s50", bufs=1)
                nc.tensor.transpose(s5i[:, 256:256 + BL], idxs[:],
                                    ident[0:BL, 0:BL])
                nc.vector.tensor_copy(idxsT[:], s5i[:, 256:256 + BL])
                for b in range(BL):
                    s5 = s3buf(b)
                    pbx = sc.tile([128, 128], fp32, tag="pbx", bufs=2)
                    nc.vector.tensor_tensor(pbx[:], predsHLR[:, 0, :, b],
                                            predsHLR[:, 1, :, b], op=ALU.add)
                    tph = s5[:, 0:128]
                    nc.tensor.transpose(tph, pbx[:], ident[:])
                    pb = sc.tile([128, 128], fp32, tag="pb", bufs=2)
                    nc.scalar.activation(pb[:], tph, AF.Copy)
                    oh = sc.tile([128, N], fp32, tag="oh", bufs=2)
                    nc.vector.tensor_scalar(oh[:], iota[:], idxsT[:, b:b + 1], None,
                                            op0=ALU.is_equal)
                    opt = s5[:, 256:384]
                    nc.tensor.matmul(opt, oh[:], pb[:], start=True, stop=True)
                    ob = sc.tile([N, D], fp32, tag="ob", bufs=2)
                    nc.scalar.activation(ob[:], opt, AF.Copy)
                    nc.sync.dma_start(out=d_out[b], in_=ob[:])

            nc.sync.dma_start(out=d_idx[:], in_=idxs[:])

    nc.finalize()
    return nc


def _prep_w(W):
    # torch-Linear weight [M_out, K_in] -> stationary lhsT sbuf layout
    # [128, K/128, M]:  sb[p, k, m] = W[m, k*128+p]
    M, K = W.shape
    kk = K // 128
    return np.ascontiguousarray(
        W.T.reshape(kk, 128, M).transpose(1, 0, 2).reshape(128, kk * M)
    ).astype(np.float32)


def _split16(a):
    hi = a.astype(np.float16)
    lo = (a - hi.astype(np.float32)).astype(np.float16)
    return hi, lo


def _prep_bias_cols(b):
    # [M] -> [128, M/128] with col j = b[j*128:(j+1)*128]
    return np.ascontiguousarray(b.reshape(-1, 128).T).astype(np.float32)


def prepare_in_maps(emb, node_emb_encoded, W1, b1, W2, b2,
                    Wih0, Whh0, bih0, bhh0, Wih1, Whh1, bih1, bhh1,
                    Wo, bo):
    bg0 = _prep_bias_cols(np.asarray(bih0) + np.asarray(bhh0))  # [128, 16]
    bg1 = _prep_bias_cols(np.asarray(bih1) + np.asarray(bhh1))
    shared = {
        "W1T": _prep_w(np.asarray(W1)).reshape(128, 4, H),
        "W2T": _prep_w(np.asarray(W2)).reshape(128, 8 * H),
        "b1c": _prep_bias_cols(np.asarray(b1)),
        "b2c": _prep_bias_cols(np.asarray(b2)),
        "bgx0": np.ascontiguousarray(
            np.repeat(bg0[:, :, None], BL, axis=2).reshape(128, 16 * BL)),
        "bgx1": np.ascontiguousarray(
            np.repeat(bg1[:, :, None], BL, axis=2).reshape(128, 16 * BL)),
        "boc": np.asarray(bo).reshape(128, 1).astype(np.float32),
        "iota128": np.tile(np.arange(N, dtype=np.float32), (128, 1)),
    }
    for nm, W in [("Wih0", Wih0), ("Whh0", Whh0), ("Wih1", Wih1),
                  ("Whh1", Whh1), ("Wo", Wo)]:
        hi, lo = _split16(_prep_w(np.asarray(W)))
        shared[nm + "H"] = hi
        shared[nm + "L"] = lo
    in_maps = []
    for c in range(NCORES):
        sl = slice(c * BL, (c + 1) * BL)
        emb_sl = np.asarray(emb[sl], dtype=np.float32)
        node_sl = np.asarray(node_emb_encoded[sl], dtype=np.float32)
        m = dict(shared)
        m["embT"] = np.ascontiguousarray(
            emb_sl.T.reshape(4, 128, BL).transpose(1, 0, 2))
        nT = np.ascontiguousarray(node_sl.transpose(2, 0, 1))
        nh, nl = _split16(nT)
        m["nodeH"] = nh
        m["nodeL"] = nl
        m["biasBN"] = np.ascontiguousarray(
            (-0.5 * (node_sl.astype(np.float64) ** 2).sum(-1)).astype(np.float32))
        in_maps.append(m)
    return in_maps


def run(inputs, reps=1, debug=False):
    from concourse.bass_utils import run_bass_kernel_spmd
    key = (reps, debug)
    if key not in _CACHE:
        _CACHE[key] = _build(reps=reps, debug=debug)
    nc = _CACHE[key]
    in_maps = prepare_in_maps(
        inputs["emb"], inputs["node_emb_encoded"], inputs["W1"], inputs["b1"],
        inputs["W2"], inputs["b2"], inputs["Wih0"], inputs["Whh0"],
        inputs["bih0"], inputs["bhh0"], inputs["Wih1"], inputs["Whh1"],
        inputs["bih1"], inputs["bhh1"], inputs["Wo"], inputs["bo"])
    res = run_bass_kernel_spmd(nc, in_maps, list(range(NCORES)))
    return res.results


def kernel(**inputs) -> np.ndarray:
    results = run(inputs, reps=1, debug=False)
    out = np.concatenate([r["outT"] for r in results], axis=0)
    return out.astype(np.float32)


# revision 3
# speedup vs baseline: 4.9708x; 1.2880x over previous
"""Trainium2 Bass kernel v3 for nn_NodeEmbDecoder (LSTM decoder + masked NN).

Data-parallel over batch B=256 across 8 cores (BL=32 rows each); weights
replicated in SBUF. Activations transposed: [feature->partitions,
batch->free]; 128x128 weight tiles stationary.

Core trick vs v2: fp32 matmuls at moving-width 32 cost ~437ns on TRN2
(stationary reload dominates) while bf16/fp16 cost ~37ns, and a 64-wide
moving operand costs the same as 32. So every fp32 gate matmul becomes
two fp16 matmuls via an error-compensated hi/lo split:

    x @ W ~= [x_hi | x_lo] @ W_hi   (one 64-col matmul, two products)
           +  x_hi @ W_lo           (one 32-col matmul, same PSUM cols)

The dropped x_lo@W_lo term is ~2^-22 relative; measured end-to-end pred
error 6.3e-7 == the fp32 baseline's, with 0/32768 masked-argmin flips.
The elementwise/activation tail, FNN init, and the output permutation
stay fp32.

Structure per step (PE program order, software-pipelined so PE never
waits on the act/elementwise tail):
    [L0ih_t][L1hh_t][L1ih_t][L0hh_{t+1}][Wo_t]
Gate activations batched [128,4*BL] per gate function; biases pre-added
on DVE from host-expanded tiles. Scores/argmax/permute as in v2 (PE
transpose + [t][b][n] DRAM staging, single-descriptor DMAs).
"""

import numpy as np

B, E, D, H, L, N = 256, 512, 128, 512, 2, 128
NCORES = 8
BL = B // NCORES  # 32 batch rows per core
UNROLL = 8

_CACHE = {}


def _build(reps=1, lsteps=N, debug=False):
    import concourse.bass as bass
    import concourse.bacc as bacc
    import concourse.tile as tile
    from concourse import mybir
    from concourse.bass import ds

    fp32 = mybir.dt.float32
    fp16 = mybir.dt.float16
    AF = mybir.ActivationFunctionType
    ALU = mybir.AluOpType

    nc = bacc.Bacc(None, target_bir_lowering=False, debug=True)

    # ---- DRAM I/O (per-core slices, host pre-laid-out) ----
    d_embT = nc.dram_tensor("embT", [128, 4, BL], fp32, kind="ExternalInput")
    d_nodeH = nc.dram_tensor("nodeH", [128, BL, N], fp16, kind="ExternalInput")
    d_nodeL = nc.dram_tensor("nodeL", [128, BL, N], fp16, kind="ExternalInput")
    d_W1 = nc.dram_tensor("W1T", [128, 4 * H], fp32, kind="ExternalInput")
    d_W2 = nc.dram_tensor("W2T", [128, 4 * 2 * H], fp32, kind="ExternalInput")
    dw16 = {}
    for nm, cols in [("Wih0", 1 * 4 * H), ("Whh0", 4 * 4 * H),
                     ("Wih1", 4 * 4 * H), ("Whh1", 4 * 4 * H), ("Wo", 4 * D)]:
        for hl in "HL":
            dw16[nm + hl] = nc.dram_tensor(nm + hl, [128, cols], fp16,
                                           kind="ExternalInput")
    d_b1 = nc.dram_tensor("b1c", [128, 4], fp32, kind="ExternalInput")
    d_b2 = nc.dram_tensor("b2c", [128, 8], fp32, kind="ExternalInput")
    d_bgx0 = nc.dram_tensor("bgx0", [128, 16 * BL], fp32, kind="ExternalInput")
    d_bgx1 = nc.dram_tensor("bgx1", [128, 16 * BL], fp32, kind="ExternalInput")
    d_bo = nc.dram_tensor("boc", [128, 1], fp32, kind="ExternalInput")
    d_biasBN = nc.dram_tensor("biasBN", [BL, N], fp32, kind="ExternalInput")
    d_iota = nc.dram_tensor("iota128", [128, N], fp32, kind="ExternalInput")

    d_out = nc.dram_tensor("outT", [BL, N, D], fp32, kind="ExternalOutput")
    d_idx = nc.dram_tensor("idxs", [BL, N], fp32, kind="ExternalOutput")

    NBLK = 32  # score-block width in steps
    n_blocks = lsteps // NBLK if lsteps >= NBLK else 0

    with tile.TileContext(nc) as tc:
        import contextlib

        with contextlib.ExitStack() as ctx:
            wp = ctx.enter_context(tc.tile_pool(name="wp", bufs=1))
            st = ctx.enter_context(tc.tile_pool(name="st", bufs=1))
            pg = ctx.enter_context(tc.tile_pool(name="pg", bufs=1, space="PSUM"))
            ps2 = ctx.enter_context(tc.tile_pool(name="ps2", bufs=2, space="PSUM"))
            gs = ctx.enter_context(tc.tile_pool(name="gs", bufs=2))
            sc = ctx.enter_context(tc.tile_pool(name="sc", bufs=4))
            dr = ctx.enter_context(tc.tile_pool(name="dr", bufs=1, space="DRAM"))

            # ---- weights / constants into SBUF (once) ----
            W1 = wp.tile([128, 4, H], fp32)
            W2 = wp.tile([128, 4, 2 * H], fp32)
            w16 = {}
            for nm, kk in [("Wih0", 1), ("Whh0", 4), ("Wih1", 4), ("Whh1", 4)]:
                for hl in "HL":
                    w16[nm + hl] = wp.tile([128, kk, 4 * H], fp16,
                                           name="sb" + nm + hl)
            for hl in "HL":
                w16["Wo" + hl] = wp.tile([128, 4, D], fp16, name="sbWo" + hl)
            embT = wp.tile([128, 4, BL], fp32)
            nodeH = wp.tile([128, BL, N], fp16)
            nodeL = wp.tile([128, BL, N], fp16)
            b1c = wp.tile([128, 4], fp32)
            b2c = wp.tile([128, 8], fp32)
            bgx0 = wp.tile([128, 4, 4, BL], fp32)
            bgx1 = wp.tile([128, 4, 4, BL], fp32)
            boc = wp.tile([128, 1], fp32)
            biasBN = wp.tile([BL, N], fp32)
            iota = wp.tile([128, N], fp32)
            loads = [
                (W1, d_W1), (W2, d_W2), (embT, d_embT), (nodeH, d_nodeH),
                (nodeL, d_nodeL), (b1c, d_b1), (b2c, d_b2), (bgx0, d_bgx0),
                (bgx1, d_bgx1), (boc, d_bo), (biasBN, d_biasBN), (iota, d_iota),
            ] + [(w16[k], dw16[k]) for k in w16]
            for dst, src in loads:
                nc.sync.dma_start(out=dst[:], in_=src[:])

            # persistent state
            h0S = st.tile([128, 4, BL], fp32)
            h1S = st.tile([128, 4, BL], fp32)
            c0S = st.tile([128, 4, BL], fp32)
            c1S = st.tile([128, 4, BL], fp32)
            xS = st.tile([128, BL], fp32)
            rT = st.tile([128, 4, BL], fp32)
            # hi/lo fp16 splits of matmul operands (hl-major free layout)
            h0HL = st.tile([128, 4, 2, BL], fp16)
            h1HL = st.tile([128, 4, 2, BL], fp16)
            xHL = st.tile([128, 2, BL], fp16)
            predsHL = st.tile([128, 2, N * BL], fp16)  # [d, hl, t*BL + b]
            scoresQ = dr.tile([N, BL * N], fp32)  # DRAM scratch [t][b*N + n]
            msk = st.tile([BL, N], fp32)
            idxs = st.tile([BL, N], fp32)
            ident = st.tile([128, 128], fp32)
            ident16 = st.tile([128, 128], fp16)
            idxsT = st.tile([128, BL], fp32)
            mx8 = st.tile([BL, 8], fp32)
            ix8 = st.tile([BL, 8], mybir.dt.uint32)
            mtile = st.tile([BL, N], fp32)
            eqm = st.tile([BL, N], fp32)

            from concourse.masks import make_identity
            make_identity(nc, ident[:])
            nc.scalar.activation(ident16[:], ident[:], AF.Copy)

            predsHLR = predsHL[:].rearrange("p h (t b) -> p h t b", b=BL)
            hS = [h0S, h1S]
            cS = [c0S, c1S]
            hHL = [h0HL, h1HL]
            WihH = [w16["Wih0H"], w16["Wih1H"]]
            WihL = [w16["Wih0L"], w16["Wih1L"]]
            WhhH = [w16["Whh0H"], w16["Whh1H"]]
            WhhL = [w16["Whh0L"], w16["Whh1L"]]
            bgxS = [bgx0, bgx1]
            nih = [1, 4]

            def split16(dst_hl, src, tmp_tag, shape_cols):
                # dst_hl[...,0,:]=fp16(src); dst_hl[...,1,:]=fp16(src-hi)
                hi = dst_hl[:, 0, :] if len(dst_hl.shape) == 3 else dst_hl[:, :, 0, :]
                lo = dst_hl[:, 1, :] if len(dst_hl.shape) == 3 else dst_hl[:, :, 1, :]
                nc.scalar.activation(hi, src, AF.Copy)
                nc.vector.tensor_tensor(lo, src, hi, op=ALU.subtract)

            pend = {0: None, 1: None}

            def gates_hh(l):
                # One merged accumulation group per 2KB PSUM bank (start=True
                # zeroes the whole bank): bank A holds gates 0-1, bank B 2-3.
                pt = pg.tile([128, 4, 4, 2 * BL], fp32, tag=f"P{l}")
                for g in range(4):
                    for j in range(4):
                        m = 4 * g + j
                        sl = slice(m * 128, (m + 1) * 128)
                        for kk in range(4):
                            mv = hHL[l][:, kk, :, :]
                            st_ = (g in (0, 2) and j == 0 and kk == 0)
                            nc.tensor.matmul(pt[:, g, j, :], WhhH[l][:, kk, sl],
                                             mv, start=st_, stop=False)
                            nc.tensor.matmul(pt[:, g, j, :], WhhL[l][:, kk, sl],
                                             mv, start=False, stop=False)
                pend[l] = pt

            def gates_ih_act(l):
                # finish bank groups with ih chunks, fold lo columns, bias, act
                pt = pend[l]
                for g in range(4):
                    for j in range(4):
                        m = 4 * g + j
                        sl = slice(m * 128, (m + 1) * 128)
                        for kk in range(nih[l]):
                            mv = xHL[:, :, :] if l == 0 else hHL[0][:, kk, :, :]
                            last = (g in (1, 3) and j == 3 and kk == nih[l] - 1)
                            nc.tensor.matmul(pt[:, g, j, :], WihH[l][:, kk, sl],
                                             mv, start=False, stop=False)
                            nc.tensor.matmul(pt[:, g, j, :], WihL[l][:, kk, sl],
                                             mv, start=False, stop=last)
                # TensorTensor may read at most one PSUM input: route the
                # hi+lo fold through an SBUF tile, folding the bias in too.
                gt = gs.tile([128, 4, 4, BL], fp32, tag=f"gt{l}")
                nc.vector.tensor_tensor(gt[:], pt[:, :, :, 0:BL], bgxS[l][:],
                                        op=ALU.add)
                nc.vector.tensor_tensor(gt[:], gt[:], pt[:, :, :, BL:2 * BL],
                                        op=ALU.add)
                acts = []
                for g in range(4):
                    a = gs.tile([128, 4, BL], fp32, tag=f"G{l}{g}")
                    fn = AF.Tanh if g == 2 else AF.Sigmoid
                    nc.scalar.activation(a[:], gt[:, g, :, :], fn)
                    acts.append(a)
                return acts

            def lstm_tail(l):
                acts = gates_ih_act(l)
                gi, gf, gg, go = acts
                t1 = gs.tile([128, 4, BL], fp32, tag=f"t1{l}")
                nc.vector.tensor_tensor(t1[:], gi[:], gg[:], op=ALU.mult)
                t2 = gs.tile([128, 4, BL], fp32, tag=f"t2{l}")
                nc.vector.tensor_tensor(t2[:], gf[:], cS[l][:], op=ALU.mult)
                nc.vector.tensor_tensor(cS[l][:], t1[:], t2[:], op=ALU.add)
                tct = gs.tile([128, 4, BL], fp32, tag=f"tc{l}")
                nc.scalar.activation(tct[:], cS[l][:], AF.Tanh)
                nc.vector.tensor_tensor(hS[l][:], go[:], tct[:], op=ALU.mult)
                split16(hHL[l][:], hS[l][:], f"s{l}", [128, 4, BL])

            def wo_step(col_ap):
                # hi and lo products accumulate into the same PSUM columns
                # (4 narrow matmuls per chunk) so no fold is needed.
                pt = ps2.tile([128, BL], fp32, tag="wo", bufs=1)
                for kk in range(4):
                    for wk, hl in [("WoH", 0), ("WoH", 1), ("WoL", 0), ("WoL", 1)]:
                        nc.tensor.matmul(pt[:], w16[wk][:, kk, :],
                                         h1HL[:, kk, hl, :],
                                         start=(kk == 0 and wk == "WoH" and hl == 0),
                                         stop=(kk == 3 and wk == "WoL" and hl == 1))
                nc.scalar.activation(xS[:], pt[:], AF.Identity, bias=boc[:])
                split16(xHL[:], xS[:], "sx", [128, BL])
                nc.vector.tensor_copy(predsHL[:, :, ds(col_ap, BL)], xHL[:])

            import contextlib as _cl
            _loop = tc.For_i(0, reps, 1) if reps > 1 else _cl.nullcontext()
            with _loop:
                nc.vector.memset(c0S[:], 0.0)
                nc.vector.memset(c1S[:], 0.0)
                nc.vector.memset(xS[:], 0.0)
                nc.vector.memset(xHL[:], 0.0)
                nc.vector.tensor_copy(msk[:], biasBN[:])

                # ---- stage 1: FNN_in -> h0, h1 (fp32) ----
                def s3buf(i):
                    return ps2.tile([128, 512], fp32, tag=f"s3{i % 2}", bufs=1,
                                    name=f"s3buf{i % 2}")

                for m in range(4):
                    pt = s3buf(m)[:, 0:BL]
                    for k in range(4):
                        nc.tensor.matmul(pt, W1[:, k, m * 128:(m + 1) * 128],
                                         embT[:, k, :], start=(k == 0), stop=(k == 3))
                    nc.scalar.activation(rT[:, m, :], pt, AF.Relu, bias=b1c[:, m:m + 1])
                for m in range(8):
                    pt = s3buf(m)[:, 0:BL]
                    for k in range(4):
                        nc.tensor.matmul(pt, W2[:, k, m * 128:(m + 1) * 128],
                                         rT[:, k, :], start=(k == 0), stop=(k == 3))
                    dst = h0S[:, m, :] if m < 4 else h1S[:, m - 4, :]
                    nc.scalar.activation(dst, pt, AF.Identity, bias=b2c[:, m:m + 1])
                split16(h0HL[:], h0S[:], "s0", [128, 4, BL])
                split16(h1HL[:], h1S[:], "s1", [128, 4, BL])

                # ---- stage 2: LSTM decode, software-pipelined ----
                with tc.For_i(0, lsteps, UNROLL) as t0:
                    gates_hh(0)
                    for k in range(UNROLL):
                        lstm_tail(0)
                        gates_hh(1)
                        lstm_tail(1)
                        if k < UNROLL - 1:
                            gates_hh(0)
                        wo_step(t0 * BL + k * BL)

                # ---- stage 3: score blocks -> DRAM in [t][b][n] layout ----
                for j in range(n_blocks):
                    tsl = slice(NBLK * j, NBLK * (j + 1))
                    for b in range(BL):
                        s3 = s3buf(b)
                        psc = s3[:, 0:NBLK]
                        for nk, hl in [("H", 0), ("H", 1), ("L", 0), ("L", 1)]:
                            nd = nodeH if nk == "H" else nodeL
                            nc.tensor.matmul(psc, nd[:, b, :],
                                             predsHLR[:, hl, tsl, b],
                                             start=(nk == "H" and hl == 0),
                                             stop=(nk == "L" and hl == 1))
                        ssb = sc.tile([128, NBLK], fp32, tag="ssb")
                        nc.vector.tensor_copy(ssb[:], psc)
                        pst = s3[0:NBLK, 2 * NBLK:2 * NBLK + N]
                        nc.tensor.transpose(pst, ssb[:], ident[:])
                        stb = sc.tile([NBLK, N], fp32, tag="stb")
                        nc.scalar.activation(stb[:], pst, AF.Copy)
                        nc.sync.dma_start(
                            out=scoresQ[tsl, N * b:N * (b + 1)],
                            in_=stb[:])

                # ---- stage 4: sequential masked argmax over n ----
                for t in range(lsteps if n_blocks else 0):
                    rd = sc.tile([BL, N], fp32, tag="rd")
                    nc.sync.dma_start(out=rd[:], in_=scoresQ[t:t + 1, :])
                    nc.vector.tensor_tensor(mtile[:], rd[:], msk[:], op=ALU.add)
                    nc.vector.max(mx8[:], mtile[:])
                    nc.vector.max_index(ix8[:], mx8[:], mtile[:])
                    nc.vector.tensor_copy(idxs[:, t:t + 1], ix8[:, 0:1])
                    nc.vector.tensor_scalar(eqm[:], iota[0:BL, :], idxs[:, t:t + 1],
                                            -1e30, op0=ALU.is_equal, op1=ALU.mult)
                    nc.vector.tensor_tensor(msk[:], msk[:], eqm[:], op=ALU.add)

                # ---- stage 5: permute preds into output slots (fp32) ----
                s5i = ps2.tile([128, 512], fp32, tag="s50", bufs=1)
                nc.tensor.transpose(s5i[:, 256:256 + BL], idxs[:],
                                    ident[0:BL, 0:BL])
                nc.vector.tensor_copy(idxsT[:], s5i[:, 256:256 + BL])
                for b in range(BL):
                    s5 = s3buf(b)
                    pbx = sc.tile([128, 128], fp32, tag="pbx", bufs=2)
                    nc.vector.tensor_tensor(pbx[:], predsHLR[:, 0, :, b],
                                            predsHLR[:, 1, :, b], op=ALU.add)
                    tph = s5[:, 0:128]
                    nc.tensor.transpose(tph, pbx[:], ident[:])
                    pb = sc.tile([128, 128], fp32, tag="pb", bufs=2)
                    nc.scalar.activation(pb[:], tph, AF.Copy)
                    oh = sc.tile([128, N], fp32, tag="oh", bufs=2)
                    nc.vector.tensor_scalar(oh[:], iota[:], idxsT[:, b:b + 1], None,
                                            op0=ALU.is_equal)
                    opt = s5[:, 256:384]
                    nc.tensor.matmul(opt, oh[:], pb[:], start=True, stop=True)
                    ob = sc.tile([N, D], fp32, tag="ob", bufs=2)
                    nc.scalar.activation(ob[:], opt, AF.Copy)
                    nc.sync.dma_start(out=d_out[b], in_=ob[:])

            nc.sync.dma_start(out=d_idx[:], in_=idxs[:])

    nc.finalize()
    return nc


def _prep_w(W):
    # torch-Linear weight [M_out, K_in] -> stationary lhsT sbuf layout
    # [128, K/128, M]:  sb[p, k, m] = W[m, k*128+p]
    M, K = W.shape
    kk = K // 128
    return np.ascontiguousarray(
        W.T.reshape(kk, 128, M).transpose(1, 0, 2).reshape(128, kk * M)
    ).astype(np.float32)


def _split16(a):
    hi = a.astype(np.float16)
    lo = (a - hi.astype(np.float32)).astype(np.float16)
    return hi, lo


def _prep_bias_cols(b):
    # [M] -> [128, M/128] with col j = b[j*128:(j+1)*128]
    return np.ascontiguousarray(b.reshape(-1, 128).T).astype(np.float32)


def prepare_in_maps(emb, node_emb_encoded, W1, b1, W2, b2,
                    Wih0, Whh0, bih0, bhh0, Wih1, Whh1, bih1, bhh1,
                    Wo, bo):
    bg0 = _prep_bias_cols(np.asarray(bih0) + np.asarray(bhh0))  # [128, 16]
    bg1 = _prep_bias_cols(np.asarray(bih1) + np.asarray(bhh1))
    shared = {
        "W1T": _prep_w(np.asarray(W1)).reshape(128, 4, H),
        "W2T": _prep_w(np.asarray(W2)).reshape(128, 8 * H),
        "b1c": _prep_bias_cols(np.asarray(b1)),
        "b2c": _prep_bias_cols(np.asarray(b2)),
        "bgx0": np.ascontiguousarray(
            np.repeat(bg0[:, :, None], BL, axis=2).reshape(128, 16 * BL)),
        "bgx1": np.ascontiguousarray(
            np.repeat(bg1[:, :, None], BL, axis=2).reshape(128, 16 * BL)),
        "boc": np.asarray(bo).reshape(128, 1).astype(np.float32),
        "iota128": np.tile(np.arange(N, dtype=np.float32), (128, 1)),
    }
    for nm, W in [("Wih0", Wih0), ("Whh0", Whh0), ("Wih1", Wih1),
                  ("Whh1", Whh1), ("Wo", Wo)]:
        hi, lo = _split16(_prep_w(np.asarray(W)))
        shared[nm + "H"] = hi
        shared[nm + "L"] = lo
    in_maps = []
    for c in range(NCORES):
        sl = slice(c * BL, (c + 1) * BL)
        emb_sl = np.asarray(emb[sl], dtype=np.float32)
        node_sl = np.asarray(node_emb_encoded[sl], dtype=np.float32)
        m = dict(shared)
        m["embT"] = np.ascontiguousarray(
            emb_sl.T.reshape(4, 128, BL).transpose(1, 0, 2))
        nT = np.ascontiguousarray(node_sl.transpose(2, 0, 1))
        nh, nl = _split16(nT)
        m["nodeH"] = nh
        m["nodeL"] = nl
        m["biasBN"] = np.ascontiguousarray(
            (-0.5 * (node_sl.astype(np.float64) ** 2).sum(-1)).astype(np.float32))
        in_maps.append(m)
    return in_maps


def run(inputs, reps=1, debug=False):
    from concourse.bass_utils import run_bass_kernel_spmd
    key = (reps, debug)
    if key not in _CACHE:
        _CACHE[key] = _build(reps=reps, debug=debug)
    nc = _CACHE[key]
    in_maps = prepare_in_maps(
        inputs["emb"], inputs["node_emb_encoded"], inputs["W1"], inputs["b1"],
        inputs["W2"], inputs["b2"], inputs["Wih0"], inputs["Whh0"],
        inputs["bih0"], inputs["bhh0"], inputs["Wih1"], inputs["Whh1"],
        inputs["bih1"], inputs["bhh1"], inputs["Wo"], inputs["bo"])
    res = run_bass_kernel_spmd(nc, in_maps, list(range(NCORES)))
    return res.results


def kernel(**inputs) -> np.ndarray:
    results = run(inputs, reps=1, debug=False)
    out = np.concatenate([r["outT"] for r in results], axis=0)
    return out.astype(np.float32)
